# revision 9
# baseline (speedup 1.0000x reference)
"""Trainium2 Bass kernel for nn_ContConv1dDenseSim (banded continuous conv).

Math (reference):
  dt[b,l,j] = times[b,l]-times[b,j], masked to a causal band j in [l-W+1, l]
  (W = (sim_size+1)*kernel_size = 30), true_ids[b,j], and a row-validity mask.
  h = relu(dt*w1+b1)  (8 hidden), kv = (h@w2+b2) masked, reshaped (16,16)
  out[b,l,o] = sum_{j,i} features[b,j,i] * kv[b,l,j,i,o]

Factorization (per core = one (batch, 128-row block), window of 157 keys):
  pre_k[j,l] = w1[k]*dt[l,j] + b1[k] is affine in (t_row[l], t_win[j]), so it
  is a rank-2 outer product: the host packs R[0,k*128+l] = w1k*t_row[l]+b1k
  and R[1,k*128+l] = -w1k, and the PE emits pre for all k with [1;t_win]
  stationary (fp16 streams at 1 col/cyc vs 4 for fp32).
  H_k = max(pre_k,0)*band in one scalar_tensor_tensor per k (PSUM->fp16).
  H_8 = band itself; the row-validity mask is folded into band on the host;
  true_ids folds into the features window.
  G[j,16k+o] = sum_i f'[j,i]*w2[k,16i+o] (2 matmuls, W2r moving), block 8 = b2.
  outT[o,l] += sum_j G[j,16k+o]*H_k[j,l]: 18 matmuls (9 channels x 2 window
  folds) with the 16-col G slice stationary (tiny LDWEIGHTS), all accumulating
  into one PSUM tile. Host transposes the (16,128) result back.

Window fold: jA = window cols 0:128 on 128 partitions, jB = cols 128:157 on
partitions 0:29 in a second 128-col block.

NOTE: TRN2 engine instructions only encode a single sync-wait; the program is
ordered so each instruction has at most one new cross-engine dependency
(observer ops where needed), and the Tile kernel-tail drain is pre-satisfied
by single-wait SP nops.
"""

import numpy as np
import concourse.bass as bass
import concourse.tile as tile
import concourse.mybir as mybir
from concourse.bass_utils import run_bass_kernel_spmd
from concourse.tile_rust import add_dep_helper

F32 = mybir.dt.float32
F16 = mybir.dt.float16
Alu = mybir.AluOpType

BS, L, CH, HID, KS = 2, 512, 16, 8, 5
LBLK = 128                      # query rows per core
NBLK = L // LBLK                # 4
NCORES = BS * NBLK              # 8
NKP = HID + 1                   # channels (8 hidden + mask/b2)
NF = NKP * CH                   # 144 G columns
RCOLS = HID * LBLK              # 1024 R columns
FT0 = 160 + RCOLS               # f'^T columns start (1184)
W20 = FT0 + 160                 # W2r columns start (1344)
PKC = W20 + NF                  # packed tensor columns (1488)

# test harness hooks
TRACE = False
LAST = None

_prog_cache = {}


def _build(W):
    WIN = LBLK + W - 1          # 157
    LO = WIN - 128              # 29
    nc = bass.Bass(trn_type="TRN2")

    pk_d = nc.declare_dram_parameter("pk", [CH, PKC], F16, isOutput=False)
    band_d = nc.declare_dram_parameter("band", [128, 256], F16, isOutput=False)
    out_d = nc.declare_dram_parameter("out", [CH, LBLK], F32, isOutput=True)

    with tile.TileContext(nc) as tc:
        with (
            tc.tile_pool(name="sb", bufs=1) as sb,
            tc.tile_pool(name="ps", bufs=1, space="PSUM") as ps,
        ):
            t_pk = sb.tile([CH, PKC], F16)
            h3 = sb.tile([128, NKP, 256], F16)
            g_sa = sb.tile([128, NF], F16)
            g_sb2 = sb.tile([LO, NF], F16)
            obs = sb.tile([1, 4], F16)

            # ---- DMAs: packed params first (PE-critical), band parallel ----
            dma_pk = nc.sync.dma_start(t_pk[:], pk_d[:])
            dma_band = nc.scalar.dma_start(h3[:, HID:NKP, :], band_d[:])

            fta = t_pk[:, FT0:FT0 + LBLK]
            ftb = t_pk[:, FT0 + LBLK:FT0 + WIN]
            w2r = t_pk[:, W20:W20 + NF]
            la = t_pk[0:2, 0:LBLK]
            lb = t_pk[0:2, LBLK:WIN]

            # ---- G = f'^T @ W2r: (128,144) + (29,144) into one bank ----
            g_ps = ps.tile([128, 2 * NF], F32)
            nc.tensor.matmul(g_ps[:, 0:NF], fta, w2r, start=True, stop=True)
            nc.tensor.matmul(g_ps[0:LO, NF:2 * NF], ftb, w2r,
                             start=True, stop=True)

            # ---- pre_k for all k: 8 rank-2 matmuls (4 k-pair PSUM tiles,
            # layout [128, 2, 256]: dim1=0 jA pair, dim1=1 jB pair),
            # interleaved A/B per tile so the DVE chain starts early ----
            pre = [ps.tile([128, 2, 256], F32, name=f"pre{q}")
                   for q in range(4)]
            for q in range(4):
                r_q = t_pk[0:2, 160 + q * 256:160 + (q + 1) * 256]
                nc.tensor.matmul(pre[q][:, 0:1, :], la, r_q,
                                 start=True, stop=True)
                nc.tensor.matmul(pre[q][0:LO, 1:2, :], lb, r_q,
                                 start=True, stop=True)

            # ---- observer (single-wait discipline) ----
            nc.vector.tensor_copy(obs[:, 0:1], h3[0:1, HID:HID + 1, 0:1])

            # ---- H_k = max(pre_k, 0) * band, one stt per k ----
            # GPSIMD cannot read PSUM, so every PSUM consumer lives on DVE.
            cp_ga = nc.vector.tensor_copy(g_sa[:], g_ps[:, 0:NF])
            cp_gb = nc.vector.tensor_copy(g_sb2[:], g_ps[0:LO, NF:2 * NF])
            stts = {}

            def stt(k):
                src = pre[k // 2][:, :, (k % 2) * 128:(k % 2) * 128 + 128]
                stts[k] = nc.vector.scalar_tensor_tensor(
                    h3[:, k:k + 1, :], src, 0.0, h3[:, HID:NKP, :],
                    Alu.max, Alu.mult)

            for k in range(HID):
                stt(k)

            # ---- outT[o,l] = sum_{k,j} G[j,16k+o] * H_k[j,l] ----
            outp = ps.tile([CH, LBLK], F32)
            band_nop = nc.tensor.nop(nofuse=True, hint="wait_band")
            add_dep_helper(band_nop.ins, dma_band.ins, sync=True,
                           reason="band cover for k=8 matmuls")

            def mm_a(k, start=False, stop=False):
                return nc.tensor.matmul(
                    outp[:], g_sa[:, k * CH:(k + 1) * CH],
                    h3[:, k:k + 1, 0:128], start=start, stop=stop)

            def mm_b(k, start=False, stop=False):
                return nc.tensor.matmul(
                    outp[:], g_sb2[:, k * CH:(k + 1) * CH],
                    h3[0:LO, k:k + 1, 128:256], start=start, stop=stop)

            for k in range(HID):
                mm_a(k, start=(k == 0))
                mm_b(k)
            mm_a(8)
            last_pe = mm_b(8, stop=True)

            # ---- store (host transposes back) ----
            o_sb = sb.tile([CH, LBLK], F32)
            last_dve = nc.vector.tensor_copy(o_sb[:], outp[:])
            dma_o = nc.sync.dma_start(out_d[:], o_sb[:])

            for prod in (dma_band, dma_pk, dma_o,
                         last_dve, last_pe):
                nop = nc.sync.nop(nofuse=True, hint="predrain_observer")
                add_dep_helper(nop.ins, prod.ins, sync=True,
                               reason="pre-drain single-wait observer")

    heavy = [(nm, type(i).__name__, len(i.sync_info.on_wait))
             for nm, i in nc.inst_map.items()
             if getattr(i, "sync_info", None) is not None
             and i.sync_info.on_wait
             and len(i.sync_info.on_wait) > 1
             and type(i).__name__ != "InstDrain"]
    if heavy:
        raise RuntimeError(f"multi-wait instructions would fail walrus: {heavy}")
    return nc


def kernel(times, features, lengths, true_ids, sim_size, w1, b1, w2, b2):
    global LAST
    times = np.ascontiguousarray(np.asarray(times, dtype=np.float32))
    features = np.ascontiguousarray(np.asarray(features, dtype=np.float32))
    lengths = np.asarray(lengths)
    true_ids = np.asarray(true_ids)
    sim = int(np.asarray(sim_size))
    w1 = np.asarray(w1, dtype=np.float32).reshape(-1)
    b1 = np.asarray(b1, dtype=np.float32).reshape(-1)
    w2 = np.asarray(w2, dtype=np.float32)
    b2 = np.asarray(b2, dtype=np.float32)

    W = (sim + 1) * KS
    WIN = LBLK + W - 1
    LO = WIN - 128

    if W not in _prog_cache:
        _prog_cache[W] = _build(W)
    nc = _prog_cache[W]

    # W2r[i, 16k+o] = w2[k, 16i+o]; col block 8 = b2
    w2r = np.concatenate(
        [w2.reshape(HID, CH, CH).transpose(1, 0, 2).reshape(CH, HID * CH),
         b2.reshape(CH, CH)], axis=1).astype(np.float16)

    # band fold (128, 256): cols 0:128 jA (j=row), cols 128:256 jB (j=128+row)
    jj = np.arange(128)[:, None]
    ll = np.arange(128)[None, :]
    band_a = ((jj >= ll) & (jj <= ll + (W - 1)))
    band_b = ((jj < LO) & (128 + jj >= ll) & (128 + jj <= ll + (W - 1)))

    in_maps = []
    for core in range(NCORES):
        b, blk = divmod(core, NBLK)
        l0 = blk * LBLK
        idx = np.arange(l0 - (W - 1), l0 + LBLK)
        valid = idx >= 0
        idxc = np.clip(idx, 0, L - 1)
        t_win = np.where(valid, times[b, idxc], 0.0).astype(np.float32)
        fmask = (true_ids[b, idxc] & valid).astype(np.float32)
        feat_win = features[b, idxc, :] * fmask[:, None]
        t_row = times[b, l0:l0 + LBLK].astype(np.float32)
        rvb = (np.arange(l0, l0 + LBLK) <=
               (sim + 1) * (int(lengths[b]) - 1)).astype(np.float32)

        band = np.zeros((128, 256), np.float16)
        band[:, 0:128] = (band_a * rvb[None, :]).astype(np.float16)
        band[:, 128:256] = (band_b * rvb[None, :]).astype(np.float16)

        pkt = np.zeros((CH, PKC), np.float16)
        pkt[0, :WIN] = 1.0
        pkt[1, :WIN] = t_win.astype(np.float16)
        r = (w1[:, None] * t_row[None, :] + b1[:, None]).astype(np.float16)
        pkt[0, 160:FT0] = r.reshape(-1)
        pkt[1, 160:FT0] = np.repeat(-w1.astype(np.float16), LBLK)
        pkt[:, FT0:FT0 + WIN] = feat_win.T.astype(np.float16)
        pkt[:, W20:W20 + NF] = w2r
        in_maps.append({"pk": pkt, "band": band})

    res = run_bass_kernel_spmd(nc, in_maps, core_ids=list(range(NCORES)),
                               trace=TRACE)
    LAST = res

    out = np.zeros((BS, L, CH), np.float32)
    for core in range(NCORES):
        b, blk = divmod(core, NBLK)
        out[b, blk * LBLK:(blk + 1) * LBLK, :] = res.results[core]["out"].T
    return out
